# revision 20
# baseline (speedup 1.0000x reference)
"""MoE routing gate network on 8 Trainium2 NeuronCores.

Pipeline (reference semantics):
  h1 = relu(conv1x1(x, w1, b1))        x:[16,256,64,64] -> [16,512,64,64]
  h2 = relu(conv1x1(h1, w2, b2))       -> [16,16,64,64]
  p  = avgpool4x4(h2)                  -> [16,16,16,16]
  h3 = conv1x1(p, w3, b3)              -> [16,16,16,16]
  logits = flatten(h3) @ w_gate        -> [16,4]
  top-2 softmax gates + cv^2 load-balancing loss

Sharding: data-parallel on batch, 2 images per core. The device kernel
computes h1/h2 (99.95% of FLOPs, fp16 matmuls with fp32 PSUM accumulation)
and the 4x4 pool sums; the tiny tail (h3, gate matmul on [16,4096]@[4096,4],
top-k, softmax, loss) runs on host at gather time, where the batch all-reduce
for importance/load is a trivial sum over the gathered rows.

Engine balance per core (cost model): PE ~44us of matmul; conv1 relu on ACT,
conv2 relu + pooling on DVE so neither elementwise engine exceeds the PE.
"""

import numpy as np

# ---- problem constants (hardcoded per contract; kernel.py is self-contained)
N_CORES = 8
B = 16
B_LOC = B // N_CORES          # 2 images per core
C_IN = 256                    # conv1 input channels  (2 k-tiles of 128)
C_MID = 512                   # conv1 output channels (4 m-tiles of 128)
C_OUT = 16                    # conv2 output channels
HW = 64 * 64                  # pixels per image
GP = 1024                     # pixel group per macro-step (2 PSUM banks fp32)
NG = HW // GP                 # 4 groups per image
XCH = 1024                    # leading x chunk so compute starts early
POOL = 16                     # pooled spatial size (4x4 mean pooling)

_CACHE: dict = {}
TRACE = False                 # test harness can flip this for profiling
LAST_RESULTS = None           # BassKernelResults of the most recent run


def _build_program():
    import concourse.mybir as mybir
    import concourse.tile as tile
    from concourse import bacc

    f32 = mybir.dt.float32
    cdt = mybir.dt.float16
    Relu = mybir.ActivationFunctionType.Relu
    add = mybir.AluOpType.add
    amax = mybir.AluOpType.max

    nc = bacc.Bacc(
        "TRN2", target_bir_lowering=False, debug=False, num_devices=N_CORES
    )
    # x split into three chunks per (b, kt) so the first matmuls wait on
    # 2x256KB of DMA instead of the full 8MB, and later groups' data keeps
    # streaming just ahead of compute.
    xa_d = nc.declare_dram_parameter("xa", [B_LOC, 2, 128, XCH], cdt, isOutput=False)
    xm_d = nc.declare_dram_parameter("xm", [B_LOC, 2, 128, XCH], cdt, isOutput=False)
    xb_d = nc.declare_dram_parameter(
        "xb", [B_LOC, 2, 128, HW - 2 * XCH], cdt, isOutput=False
    )
    w1t_d = nc.declare_dram_parameter("w1t", [128, 2 * C_MID], cdt, isOutput=False)
    w2t_d = nc.declare_dram_parameter("w2t", [128, 4 * C_OUT], cdt, isOutput=False)
    bias_d = nc.declare_dram_parameter("bias", [128, 5], f32, isOutput=False)
    out_d = nc.declare_dram_parameter(
        "pooled", [B_LOC, C_OUT, POOL * POOL], f32, isOutput=True
    )

    with tile.TileContext(nc) as tc:
        with (
            tc.tile_pool(name="weights", bufs=1) as wp,
            tc.tile_pool(name="xin", bufs=1) as xp,
            tc.tile_pool(name="h1", bufs=2) as h1p,
            tc.tile_pool(name="h2", bufs=2) as h2p,
            tc.tile_pool(name="pool", bufs=1) as poolp,
            tc.tile_pool(name="ps1", bufs=2, space="PSUM") as ps1p,
            tc.tile_pool(name="ps2", bufs=2, space="PSUM") as ps2p,
        ):
            # DMA issue order = first-use order: conv1 weights, the first
            # group's x, then biases/conv2 weights, then the x stream.
            w1sb = wp.tile([128, 2 * C_MID], cdt, tag="w1", name="w1sb")
            nc.sync.dma_start(w1sb[:], w1t_d[:])

            xs_a, xs_m, xs_b = {}, {}, {}
            for kt in range(2):
                t = xp.tile([128, XCH], cdt, tag=f"xa0{kt}", name=f"xa0{kt}")
                nc.sync.dma_start(t[:], xa_d[0, kt])
                xs_a[0, kt] = t

            biassb = wp.tile([128, 5], f32, tag="bias", name="biassb")
            nc.sync.dma_start(biassb[:], bias_d[:])
            b2ap = biassb[0:C_OUT, 4:5]
            w2sb = wp.tile([128, 4 * C_OUT], cdt, tag="w2", name="w2sb")
            nc.sync.dma_start(w2sb[:], w2t_d[:])

            def stream_x(b, kt, which):
                d, store, cols = {
                    "m": (xm_d, xs_m, XCH),
                    "b": (xb_d, xs_b, HW - 2 * XCH),
                }[which]
                t = xp.tile(
                    [128, cols], cdt, tag=f"x{which}{b}{kt}", name=f"x{which}{b}{kt}"
                )
                nc.sync.dma_start(t[:], d[b, kt])
                store[b, kt] = t

            for kt in range(2):
                stream_x(0, kt, "m")
            for kt in range(2):
                stream_x(0, kt, "b")
            for kt in range(2):
                t = xp.tile([128, XCH], cdt, tag=f"xa1{kt}", name=f"xa1{kt}")
                nc.sync.dma_start(t[:], xa_d[1, kt])
                xs_a[1, kt] = t
            for kt in range(2):
                stream_x(1, kt, "m")
            for kt in range(2):
                stream_x(1, kt, "b")

            def x_slice(b, kt, lo, size):
                if lo < XCH:
                    return xs_a[b, kt][:, lo : lo + size]
                if lo < 2 * XCH:
                    return xs_m[b, kt][:, lo - XCH : lo - XCH + size]
                return xs_b[b, kt][:, lo - 2 * XCH : lo - 2 * XCH + size]

            pooled_t = {
                b: poolp.tile(
                    [C_OUT, POOL * POOL], f32, tag=f"pool{b}", name=f"pool{b}"
                )
                for b in range(B_LOC)
            }

            def pool_reduce(src_ap, b, col_off, npr):
                # npr pool rows of 16 cols each; src holds npr*256 pixels
                v = src_ap.rearrange(
                    "p (pr hr pc wc) -> p pr pc hr wc", pr=npr, hr=4, pc=16, wc=4
                )
                nc.vector.tensor_reduce(
                    pooled_t[b][:, col_off : col_off + npr * 16],
                    v,
                    axis=mybir.AxisListType.XY,
                    op=add,
                )

            def emit_l1_pair(b, g, pair, h1s):
                for ot in (2 * pair, 2 * pair + 1):
                    ps1 = ps1p.tile([128, GP], f32, tag="ps1", name="ps1")
                    for half in range(2):
                        lo = g * GP + half * 512
                        for kt in range(2):
                            nc.tensor.matmul(
                                ps1[:, half * 512 : (half + 1) * 512],
                                w1sb[
                                    :,
                                    kt * C_MID + ot * 128 : kt * C_MID + (ot + 1) * 128,
                                ],
                                x_slice(b, kt, lo, 512),
                                start=(kt == 0),
                                stop=(kt == 1),
                            )
                    h1 = h1p.tile([128, GP], cdt, tag=f"h1_{ot}", name=f"h1_{ot}")
                    nc.scalar.activation(
                        h1[:], ps1[:], Relu, bias=biassb[:, ot : ot + 1]
                    )
                    h1s.append(h1)

            def emit_l2_half(b, g, h1s, half, act_copy=False):
                # conv2 as two concurrent column-group matmul chains:
                # kt0+kt1 -> PSUM partitions 0:16, kt2+kt3 -> 32:48. The PE
                # runs both streams in parallel (distinct col groups) on HW.
                # Partials are combined on DVE (re-base copy + add) and
                # relu'd on GPSIMD so no single engine exceeds the PE time.
                sl = slice(half * 512, (half + 1) * 512)
                ps2 = ps2p.tile([48, 512], f32, tag="ps2", name="ps2")
                for step in range(2):  # interleave issue for concurrency
                    for pair in range(2):
                        kt = 2 * pair + step
                        nc.tensor.matmul(
                            ps2[32 * pair : 32 * pair + C_OUT, :],
                            w2sb[:, kt * C_OUT : (kt + 1) * C_OUT],
                            h1s[kt][:, sl],
                            start=(step == 0),
                            stop=(step == 1),
                            tile_position=(0, 32 * pair),
                        )
                p23 = h2p.tile([C_OUT, 512], f32, tag="p23", name="p23")
                if act_copy:
                    # near the kernel tail ACT is idle; shift the re-base
                    # copy there so the DVE queue drains sooner
                    nc.scalar.copy(p23[:], ps2[32 : 32 + C_OUT, :])
                else:
                    nc.vector.tensor_copy(p23[:], ps2[32 : 32 + C_OUT, :])
                h2pre = h2p.tile([C_OUT, 512], f32, tag="h2pre", name="h2pre")
                nc.vector.scalar_tensor_tensor(
                    h2pre[:], ps2[0:C_OUT, :], b2ap, p23[:], op0=add, op1=add
                )
                h2 = h2p.tile([C_OUT, 512], f32, tag="h2", name="h2")
                nc.gpsimd.tensor_scalar(h2[:], h2pre[:], 0.0, None, op0=amax)
                pool_reduce(h2[:], b, g * 64 + half * 32, 2)

            def emit_l2_done(b, g):
                if g == NG - 1:
                    nc.sync.dma_start(out_d[b], pooled_t[b][:])

            # Software pipeline at half-step granularity: L2 halves of step
            # i-1 slot between the L1 ot-pairs of step i, so the PE stays
            # dense and the epilogue chain never lags more than ~half a step.
            steps = [(b, g) for b in range(B_LOC) for g in range(NG)]
            prev = None
            for i, (b, g) in enumerate(steps):
                h1s = []
                emit_l1_pair(b, g, 0, h1s)
                if prev is not None:
                    emit_l2_half(*prev, half=0, act_copy=(i >= len(steps) - 1))
                emit_l1_pair(b, g, 1, h1s)
                if prev is not None:
                    emit_l2_half(*prev, half=1, act_copy=(i >= len(steps) - 1))
                    emit_l2_done(prev[0], prev[1])
                prev = (b, g, h1s)
            emit_l2_half(*prev, half=0, act_copy=True)
            emit_l2_half(*prev, half=1, act_copy=True)
            emit_l2_done(prev[0], prev[1])

    nc.compile()
    return nc


def _run_device(x, w1, b1, w2, b2):
    global LAST_RESULTS
    from concourse.bass_utils import run_bass_kernel_spmd

    if "nc" not in _CACHE:
        _CACHE["nc"] = _build_program()
    nc = _CACHE["nc"]

    # [kt, c, o] -> [c, kt*512+o] so one DMA loads all conv1 weights
    w1t = np.ascontiguousarray(
        w1.T.reshape(2, 128, C_MID).transpose(1, 0, 2).reshape(128, 2 * C_MID)
    ).astype(np.float16)
    w2t = np.ascontiguousarray(
        w2.T.reshape(4, 128, C_OUT).transpose(1, 0, 2).reshape(128, 4 * C_OUT)
    ).astype(np.float16)
    bias = np.zeros((128, 5), np.float32)
    bias[:, :4] = b1.reshape(4, 128).T
    bias[:C_OUT, 4] = b2
    x16 = x.reshape(B, 2, 128, HW).astype(np.float16)
    xa = np.ascontiguousarray(x16[:, :, :, :XCH])
    xm = np.ascontiguousarray(x16[:, :, :, XCH : 2 * XCH])
    xb = np.ascontiguousarray(x16[:, :, :, 2 * XCH :])

    in_maps = []
    for i in range(N_CORES):
        sl = slice(B_LOC * i, B_LOC * (i + 1))
        in_maps.append(
            {
                "xa": xa[sl],
                "xm": xm[sl],
                "xb": xb[sl],
                "w1t": w1t,
                "w2t": w2t,
                "bias": bias,
            }
        )

    res = run_bass_kernel_spmd(nc, in_maps, list(range(N_CORES)), trace=TRACE)
    LAST_RESULTS = res
    pooled = np.stack([res.results[i]["pooled"] for i in range(N_CORES)])
    return pooled.reshape(B, C_OUT, POOL, POOL)


def kernel(x, w1, b1, w2, b2, w3, b3, w_gate):
    pool_sums = _run_device(
        np.asarray(x, dtype=np.float32),
        np.asarray(w1, dtype=np.float32),
        np.asarray(b1, dtype=np.float32),
        np.asarray(w2, dtype=np.float32),
        np.asarray(b2, dtype=np.float32),
    )
    pooled = pool_sums.astype(np.float64) / 16.0

    # tiny tail: conv3, gate matmul, top-2 softmax, cv^2 loss (host, fp64)
    h3 = np.einsum("oc,bchw->bohw", np.asarray(w3, np.float64), pooled)
    h3 += np.asarray(b3, np.float64)[None, :, None, None]
    feats = h3.reshape(B, -1)
    logits = feats @ np.asarray(w_gate, np.float64)

    idx = np.argsort(-logits, axis=1, kind="stable")[:, :2].astype(np.int32)
    vals = np.take_along_axis(logits, idx, axis=1)
    e = np.exp(vals - vals.max(axis=1, keepdims=True))
    gates_k = e / e.sum(axis=1, keepdims=True)

    gates = np.zeros_like(logits)
    np.put_along_axis(gates, idx, gates_k, axis=1)
    importance = gates.sum(axis=0)
    load = (gates > 0).sum(axis=0).astype(np.float64)

    def cv_squared(v):
        return v.var(ddof=1) / (v.mean() ** 2 + 1e-10)

    loss = cv_squared(importance) + cv_squared(load)

    return (
        gates_k.astype(np.float32),
        idx,
        np.float32(loss),
    )


# revision 21
# speedup vs baseline: 1.0140x; 1.0140x over previous
"""MoE routing gate network on 8 Trainium2 NeuronCores.

Pipeline (reference semantics):
  h1 = relu(conv1x1(x, w1, b1))        x:[16,256,64,64] -> [16,512,64,64]
  h2 = relu(conv1x1(h1, w2, b2))       -> [16,16,64,64]
  p  = avgpool4x4(h2)                  -> [16,16,16,16]
  h3 = conv1x1(p, w3, b3)              -> [16,16,16,16]
  logits = flatten(h3) @ w_gate        -> [16,4]
  top-2 softmax gates + cv^2 load-balancing loss

Sharding: data-parallel on batch, 2 images per core. The device kernel
computes h1/h2 (99.95% of FLOPs, fp16 matmuls with fp32 PSUM accumulation)
and the 4x4 pool sums; the tiny tail (h3, gate matmul on [16,4096]@[4096,4],
top-k, softmax, loss) runs on host at gather time, where the batch all-reduce
for importance/load is a trivial sum over the gathered rows.

Engine balance per core (cost model): PE ~44us of matmul; conv1 relu on ACT,
conv2 relu + pooling on DVE so neither elementwise engine exceeds the PE.
"""

import numpy as np

# ---- problem constants (hardcoded per contract; kernel.py is self-contained)
N_CORES = 8
B = 16
B_LOC = B // N_CORES          # 2 images per core
C_IN = 256                    # conv1 input channels  (2 k-tiles of 128)
C_MID = 512                   # conv1 output channels (4 m-tiles of 128)
C_OUT = 16                    # conv2 output channels
HW = 64 * 64                  # pixels per image
GP = 1024                     # pixel group per macro-step (2 PSUM banks fp32)
NG = HW // GP                 # 4 groups per image
XCH = 1024                    # leading x chunk so compute starts early
POOL = 16                     # pooled spatial size (4x4 mean pooling)

_CACHE: dict = {}
TRACE = False                 # test harness can flip this for profiling
LAST_RESULTS = None           # BassKernelResults of the most recent run


def _build_program():
    import concourse.mybir as mybir
    import concourse.tile as tile
    from concourse import bacc

    f32 = mybir.dt.float32
    cdt = mybir.dt.float16
    Relu = mybir.ActivationFunctionType.Relu
    add = mybir.AluOpType.add
    amax = mybir.AluOpType.max

    nc = bacc.Bacc(
        "TRN2", target_bir_lowering=False, debug=False, num_devices=N_CORES
    )
    # x split into three chunks per (b, kt) so the first matmuls wait on
    # 2x256KB of DMA instead of the full 8MB, and later groups' data keeps
    # streaming just ahead of compute.
    xa_d = nc.declare_dram_parameter("xa", [B_LOC, 2, 128, XCH], cdt, isOutput=False)
    xm_d = nc.declare_dram_parameter("xm", [B_LOC, 2, 128, XCH], cdt, isOutput=False)
    xb_d = nc.declare_dram_parameter(
        "xb", [B_LOC, 2, 128, HW - 2 * XCH], cdt, isOutput=False
    )
    w1t_d = nc.declare_dram_parameter("w1t", [128, 2 * C_MID], cdt, isOutput=False)
    w2t_d = nc.declare_dram_parameter("w2t", [128, 4 * C_OUT], cdt, isOutput=False)
    bias_d = nc.declare_dram_parameter("bias", [128, 5], f32, isOutput=False)
    out_d = nc.declare_dram_parameter(
        "pooled", [B_LOC, C_OUT, POOL * POOL], f32, isOutput=True
    )

    with tile.TileContext(nc) as tc:
        with (
            tc.tile_pool(name="weights", bufs=1) as wp,
            tc.tile_pool(name="xin", bufs=1) as xp,
            tc.tile_pool(name="h1", bufs=2) as h1p,
            tc.tile_pool(name="h2", bufs=2) as h2p,
            tc.tile_pool(name="pool", bufs=1) as poolp,
            tc.tile_pool(name="ps1", bufs=2, space="PSUM") as ps1p,
            tc.tile_pool(name="ps2", bufs=4, space="PSUM") as ps2p,
        ):
            # DMA issue order = first-use order: conv1 weights, the first
            # group's x, then biases/conv2 weights, then the x stream.
            w1sb = wp.tile([128, 2 * C_MID], cdt, tag="w1", name="w1sb")
            nc.sync.dma_start(w1sb[:], w1t_d[:])

            xs_a, xs_m, xs_b = {}, {}, {}
            for kt in range(2):
                t = xp.tile([128, XCH], cdt, tag=f"xa0{kt}", name=f"xa0{kt}")
                nc.sync.dma_start(t[:], xa_d[0, kt])
                xs_a[0, kt] = t

            biassb = wp.tile([128, 5], f32, tag="bias", name="biassb")
            nc.sync.dma_start(biassb[:], bias_d[:])
            b2ap = biassb[0:C_OUT, 4:5]
            w2sb = wp.tile([128, 4 * C_OUT], cdt, tag="w2", name="w2sb")
            nc.sync.dma_start(w2sb[:], w2t_d[:])

            def stream_x(b, kt, which):
                d, store, cols = {
                    "m": (xm_d, xs_m, XCH),
                    "b": (xb_d, xs_b, HW - 2 * XCH),
                }[which]
                t = xp.tile(
                    [128, cols], cdt, tag=f"x{which}{b}{kt}", name=f"x{which}{b}{kt}"
                )
                nc.sync.dma_start(t[:], d[b, kt])
                store[b, kt] = t

            for kt in range(2):
                stream_x(0, kt, "m")
            for kt in range(2):
                stream_x(0, kt, "b")
            for kt in range(2):
                t = xp.tile([128, XCH], cdt, tag=f"xa1{kt}", name=f"xa1{kt}")
                nc.sync.dma_start(t[:], xa_d[1, kt])
                xs_a[1, kt] = t
            for kt in range(2):
                stream_x(1, kt, "m")
            for kt in range(2):
                stream_x(1, kt, "b")

            def x_slice(b, kt, lo, size):
                if lo < XCH:
                    return xs_a[b, kt][:, lo : lo + size]
                if lo < 2 * XCH:
                    return xs_m[b, kt][:, lo - XCH : lo - XCH + size]
                return xs_b[b, kt][:, lo - 2 * XCH : lo - 2 * XCH + size]

            pooled_t = {
                b: poolp.tile(
                    [C_OUT, POOL * POOL], f32, tag=f"pool{b}", name=f"pool{b}"
                )
                for b in range(B_LOC)
            }

            def pool_reduce(src_ap, b, col_off, npr):
                # npr pool rows of 16 cols each; src holds npr*256 pixels
                v = src_ap.rearrange(
                    "p (pr hr pc wc) -> p pr pc hr wc", pr=npr, hr=4, pc=16, wc=4
                )
                nc.vector.tensor_reduce(
                    pooled_t[b][:, col_off : col_off + npr * 16],
                    v,
                    axis=mybir.AxisListType.XY,
                    op=add,
                )

            def emit_l1_pair(b, g, pair, h1s):
                for ot in (2 * pair, 2 * pair + 1):
                    ps1 = ps1p.tile([128, GP], f32, tag="ps1", name="ps1")
                    for half in range(2):
                        lo = g * GP + half * 512
                        for kt in range(2):
                            nc.tensor.matmul(
                                ps1[:, half * 512 : (half + 1) * 512],
                                w1sb[
                                    :,
                                    kt * C_MID + ot * 128 : kt * C_MID + (ot + 1) * 128,
                                ],
                                x_slice(b, kt, lo, 512),
                                start=(kt == 0),
                                stop=(kt == 1),
                            )
                    h1 = h1p.tile([128, GP], cdt, tag=f"h1_{ot}", name=f"h1_{ot}")
                    nc.scalar.activation(
                        h1[:], ps1[:], Relu, bias=biassb[:, ot : ot + 1]
                    )
                    h1s.append(h1)

            def emit_l2_half(b, g, h1s, half, act_copy=False):
                # conv2 as two concurrent column-group matmul chains:
                # kt0+kt1 -> PSUM partitions 0:16, kt2+kt3 -> 32:48. The PE
                # runs both streams in parallel (distinct col groups) on HW.
                # Partials are combined on DVE (re-base copy + add) and
                # relu'd on GPSIMD so no single engine exceeds the PE time.
                sl = slice(half * 512, (half + 1) * 512)
                ps2 = ps2p.tile([48, 512], f32, tag="ps2", name="ps2")
                for step in range(2):  # interleave issue for concurrency
                    for pair in range(2):
                        kt = 2 * pair + step
                        nc.tensor.matmul(
                            ps2[32 * pair : 32 * pair + C_OUT, :],
                            w2sb[:, kt * C_OUT : (kt + 1) * C_OUT],
                            h1s[kt][:, sl],
                            start=(step == 0),
                            stop=(step == 1),
                            tile_position=(0, 32 * pair),
                        )
                p23 = h2p.tile([C_OUT, 512], f32, tag="p23", name="p23")
                if act_copy:
                    # near the kernel tail ACT is idle; shift the re-base
                    # copy there so the DVE queue drains sooner
                    nc.scalar.copy(p23[:], ps2[32 : 32 + C_OUT, :])
                else:
                    nc.vector.tensor_copy(p23[:], ps2[32 : 32 + C_OUT, :])
                h2pre = h2p.tile([C_OUT, 512], f32, tag="h2pre", name="h2pre")
                nc.vector.scalar_tensor_tensor(
                    h2pre[:], ps2[0:C_OUT, :], b2ap, p23[:], op0=add, op1=add
                )
                h2 = h2p.tile([C_OUT, 512], f32, tag="h2", name="h2")
                nc.gpsimd.tensor_scalar(h2[:], h2pre[:], 0.0, None, op0=amax)
                pool_reduce(h2[:], b, g * 64 + half * 32, 2)

            def emit_l2_done(b, g):
                if g == NG - 1:
                    nc.sync.dma_start(out_d[b], pooled_t[b][:])

            # Software pipeline at half-step granularity: L2 halves of step
            # i-1 slot between the L1 ot-pairs of step i, so the PE stays
            # dense and the epilogue chain never lags more than ~half a step.
            steps = [(b, g) for b in range(B_LOC) for g in range(NG)]
            prev = None
            for i, (b, g) in enumerate(steps):
                h1s = []
                emit_l1_pair(b, g, 0, h1s)
                if prev is not None:
                    emit_l2_half(*prev, half=0, act_copy=(i >= len(steps) - 1))
                emit_l1_pair(b, g, 1, h1s)
                if prev is not None:
                    emit_l2_half(*prev, half=1, act_copy=(i >= len(steps) - 1))
                    emit_l2_done(prev[0], prev[1])
                prev = (b, g, h1s)
            emit_l2_half(*prev, half=0, act_copy=True)
            emit_l2_half(*prev, half=1, act_copy=True)
            emit_l2_done(prev[0], prev[1])

    nc.compile()
    return nc


def _run_device(x, w1, b1, w2, b2):
    global LAST_RESULTS
    from concourse.bass_utils import run_bass_kernel_spmd

    if "nc" not in _CACHE:
        _CACHE["nc"] = _build_program()
    nc = _CACHE["nc"]

    # [kt, c, o] -> [c, kt*512+o] so one DMA loads all conv1 weights
    w1t = np.ascontiguousarray(
        w1.T.reshape(2, 128, C_MID).transpose(1, 0, 2).reshape(128, 2 * C_MID)
    ).astype(np.float16)
    w2t = np.ascontiguousarray(
        w2.T.reshape(4, 128, C_OUT).transpose(1, 0, 2).reshape(128, 4 * C_OUT)
    ).astype(np.float16)
    bias = np.zeros((128, 5), np.float32)
    bias[:, :4] = b1.reshape(4, 128).T
    bias[:C_OUT, 4] = b2
    x16 = x.reshape(B, 2, 128, HW).astype(np.float16)
    xa = np.ascontiguousarray(x16[:, :, :, :XCH])
    xm = np.ascontiguousarray(x16[:, :, :, XCH : 2 * XCH])
    xb = np.ascontiguousarray(x16[:, :, :, 2 * XCH :])

    in_maps = []
    for i in range(N_CORES):
        sl = slice(B_LOC * i, B_LOC * (i + 1))
        in_maps.append(
            {
                "xa": xa[sl],
                "xm": xm[sl],
                "xb": xb[sl],
                "w1t": w1t,
                "w2t": w2t,
                "bias": bias,
            }
        )

    res = run_bass_kernel_spmd(nc, in_maps, list(range(N_CORES)), trace=TRACE)
    LAST_RESULTS = res
    pooled = np.stack([res.results[i]["pooled"] for i in range(N_CORES)])
    return pooled.reshape(B, C_OUT, POOL, POOL)


def kernel(x, w1, b1, w2, b2, w3, b3, w_gate):
    pool_sums = _run_device(
        np.asarray(x, dtype=np.float32),
        np.asarray(w1, dtype=np.float32),
        np.asarray(b1, dtype=np.float32),
        np.asarray(w2, dtype=np.float32),
        np.asarray(b2, dtype=np.float32),
    )
    pooled = pool_sums.astype(np.float64) / 16.0

    # tiny tail: conv3, gate matmul, top-2 softmax, cv^2 loss (host, fp64)
    h3 = np.einsum("oc,bchw->bohw", np.asarray(w3, np.float64), pooled)
    h3 += np.asarray(b3, np.float64)[None, :, None, None]
    feats = h3.reshape(B, -1)
    logits = feats @ np.asarray(w_gate, np.float64)

    idx = np.argsort(-logits, axis=1, kind="stable")[:, :2].astype(np.int32)
    vals = np.take_along_axis(logits, idx, axis=1)
    e = np.exp(vals - vals.max(axis=1, keepdims=True))
    gates_k = e / e.sum(axis=1, keepdims=True)

    gates = np.zeros_like(logits)
    np.put_along_axis(gates, idx, gates_k, axis=1)
    importance = gates.sum(axis=0)
    load = (gates > 0).sum(axis=0).astype(np.float64)

    def cv_squared(v):
        return v.var(ddof=1) / (v.mean() ** 2 + 1e-10)

    loss = cv_squared(importance) + cv_squared(load)

    return (
        gates_k.astype(np.float32),
        idx,
        np.float32(loss),
    )
